# revision 8
# baseline (speedup 1.0000x reference)
"""GAT-mod forward on 8 trn2 NeuronCores (Bass/Tile).

Strategy (dst-sharded, slot-major message passing):
- Nodes are partitioned across 8 cores by destination id (6250 each).
- Each core builds the full node table T[n] = [h(n) bf16(256) | a_src(n) f32(4) | pad]
  (768B rows) in its local HBM (recompute is cheaper than all-gather), split
  logically at row 25000 so gather indices fit int16 (dma_gather limit), with a
  PAD row per half (h=0, a_src=-1e4 -> exp underflows to exactly 0).
- Edges (incl. self-loops) are grouped by 128-node destination windows, split
  into lo/hi source halves, packed into 128-slot batches (slot = edge).
  Per batch the host emits fp8 indicator matrices Ind[slot, node] and its
  transpose; the device then:
    gather rows -> e = lrelu(a_src + IndT@a_dst) -> p = exp(e) (no-max softmax;
    e is bounded by construction so exp cannot overflow) -> msg = [g*p | p]
    -> PSUM[node, 260] += Ind^T-weighted sum via PE matmul.
  alpha normalization (p/denom) is applied after aggregation per node.
- BN batch stats via partial sums + AllReduce across the 8 cores.
"""

import os
import sys
import hashlib

import numpy as np
import ml_dtypes

N = 50000
E = 800000
D = 64
H = 4
HD = 256
NEG = 0.2
BN_EPS = 1e-5
NC = 8
SLAB = N // NC          # 6250
W = 128                 # window nodes
NW = (SLAB + W - 1) // W  # 49
LAST_ROWS = SLAB - (NW - 1) * W  # 106
LO = 25000
RE = 384                # table row elems (bf16): 256 h + 8 (4 f32 a_src) + pad
TROWS = 2 * LO + 2      # 50002 (two pad rows)
PAD_LO = LO             # pad row index within lo half
PAD_HI = LO             # within hi half (row 25001+25000 = 50001)
BPC = 8                 # batches per gather call
CALL = BPC * 128        # 1024 idxs per gather

_CACHE = {}
LAST_EXEC_NS = None
LAST_TRACE = None


def _install_ntff_shim():
    import contextlib
    import ctypes
    import types

    if "antenv.axon_hooks" in sys.modules:
        return
    so_path = "/opt/axon/libaxon_pjrt.so"

    def _hook_factory(so_path):
        try:
            lib = ctypes.CDLL(so_path)
        except OSError:
            return None
        if not hasattr(lib, "axon_start_nrt_profile"):
            return None
        lib.axon_start_nrt_profile.argtypes = [ctypes.POINTER(ctypes.c_int64), ctypes.c_size_t]
        lib.axon_start_nrt_profile.restype = ctypes.c_int64
        lib.axon_stop_nrt_profile.argtypes = [ctypes.c_char_p]
        lib.axon_stop_nrt_profile.restype = ctypes.c_int64

        @contextlib.contextmanager
        def _hook(output_dir, device_ids):
            import jax

            jax.devices()
            if device_ids:
                ids = (ctypes.c_int64 * len(device_ids))(*device_ids)
                rc = lib.axon_start_nrt_profile(ids, len(device_ids))
            else:
                rc = lib.axon_start_nrt_profile(None, 0)
            if rc != 0:
                raise RuntimeError(f"axon_start_nrt_profile rc={rc}")
            try:
                yield
            finally:
                lib.axon_stop_nrt_profile(str(output_dir).encode())

        return _hook

    mod = types.ModuleType("antenv.axon_hooks")
    _h = [None]
    mod.set_axon_ntff_profile_hook = lambda h: _h.__setitem__(0, h)
    mod.get_axon_ntff_profile_hook = lambda: _h[0]
    sys.modules["antenv.axon_hooks"] = mod
    try:
        import antenv

        antenv.axon_hooks = mod
    except ImportError:
        pass
    mod.set_axon_ntff_profile_hook(_hook_factory(so_path))


# ----------------------------------------------------------------- host prep
def _schedule_and_blobs(edge_index):
    src = np.concatenate([edge_index[0].astype(np.int64), np.arange(N, dtype=np.int64)])
    dst = np.concatenate([edge_index[1].astype(np.int64), np.arange(N, dtype=np.int64)])

    cores = []
    for c in range(NC):
        sel = (dst >= c * SLAB) & (dst < (c + 1) * SLAB)
        s_src = src[sel]
        s_dst = dst[sel] - c * SLAB
        islo = s_src < LO
        win = s_dst >> 7
        secid = win * 2 + (1 - islo.astype(np.int64))  # even = lo, odd = hi
        order = np.argsort(secid, kind="stable")
        cores.append((s_src[order], s_dst[order], secid[order]))

    # per-(core, section) counts; shared schedule = max over cores
    NSEC = NW * 2
    cnts = np.zeros((NC, NSEC), np.int64)
    for c in range(NC):
        binc = np.bincount(cores[c][2], minlength=NSEC)
        cnts[c] = binc
    nb_sec = (np.max(cnts, axis=0) + 127) // 128  # batches per section
    nb_sec = np.maximum(nb_sec, 1)

    # batch list: lo run (even sections, w ascending), then hi run
    batches = []  # (w, kind, sec, dead)
    for kind in (0, 1):  # 0=lo, 1=hi
        run_start = len(batches)
        for wdx in range(NW):
            s = wdx * 2 + kind
            for _ in range(int(nb_sec[s])):
                batches.append([wdx, kind, s, False])
        while (len(batches) - run_start) % BPC != 0:
            batches.append([0, kind, -1, True])
    NB = len(batches)
    NCALLS = NB // BPC
    call_kind = [batches[ci * BPC][1] for ci in range(NCALLS)]

    # mark section start/stop per batch
    sec_first = {}
    sec_last = {}
    for bi, (wdx, kind, s, dead) in enumerate(batches):
        if dead:
            continue
        if s not in sec_first:
            sec_first[s] = bi
        sec_last[s] = bi
    binfo = []
    for bi, (wdx, kind, s, dead) in enumerate(batches):
        binfo.append(dict(w=wdx, kind=kind, sec=s, dead=dead,
                          start=(not dead and sec_first[s] == bi),
                          stop=(not dead and sec_last[s] == bi)))

    sched = dict(NB=NB, NCALLS=NCALLS, call_kind=call_kind, binfo=binfo)

    # per-core blobs
    blobs = []
    for c in range(NC):
        s_src, s_dst, s_sec = cores[c]
        gidx = np.full((NB * 128,), PAD_LO, np.int64)
        ind = np.zeros((NB, 128, 128), np.float32)
        indt = np.zeros((NB, 128, 128), np.float32)
        # slot assignment: per section, edges fill batches in order
        sec_edge_start = np.zeros(NSEC + 1, np.int64)
        np.cumsum(np.bincount(s_sec, minlength=NSEC), out=sec_edge_start[1:])
        # batch index of each section's first batch
        sec_b0 = {}
        for bi, info in enumerate(binfo):
            if not info["dead"] and info["sec"] not in sec_b0:
                sec_b0[info["sec"]] = bi
        for s in range(NSEC):
            e0, e1 = sec_edge_start[s], sec_edge_start[s + 1]
            if e1 == e0:
                continue
            n = e1 - e0
            b0 = sec_b0[s]
            slots = b0 * 128 + np.arange(n)
            srcs = s_src[e0:e1]
            kind = s & 1
            idxv = np.where(kind == 0, srcs, srcs - LO)
            gidx[slots] = idxv
            node_in_w = (s_dst[e0:e1] - (s >> 1) * 128).astype(np.int64)
            bloc = slots // 128
            sloc = slots % 128
            ind[bloc, sloc, node_in_w] = 1.0
            indt[bloc, node_in_w, sloc] = 1.0
        # wrap gather indices: call ci covers positions [ci*1024, +1024)
        g16 = gidx.astype(np.int16).reshape(NCALLS, 64, 16)
        gw = np.transpose(g16, (0, 2, 1)).reshape(NCALLS, 16, 64)
        gw = np.tile(gw, (1, 8, 1))  # [NCALLS, 128, 64]
        GIDX = np.ascontiguousarray(np.transpose(gw, (1, 0, 2)).reshape(128, NCALLS * 64))
        INDB = np.ascontiguousarray(
            np.transpose(ind, (1, 0, 2)).reshape(128, NB * 128)).astype(ml_dtypes.float8_e4m3)
        INDTB = np.ascontiguousarray(
            np.transpose(indt, (1, 0, 2)).reshape(128, NB * 128)).astype(ml_dtypes.float8_e4m3)
        blobs.append(dict(GIDX=GIDX, INDB=INDB, INDTB=INDTB))
    return sched, blobs


def _build_program(sched):
    from concourse import bacc, mybir
    from concourse.tile import TileContext

    AL = mybir.AluOpType
    AF = mybir.ActivationFunctionType
    f32 = mybir.dt.float32
    bf16 = mybir.dt.bfloat16
    fp8 = mybir.dt.float8e4
    i16 = mybir.dt.int16

    NB = sched["NB"]
    NCALLS = sched["NCALLS"]
    binfo = sched["binfo"]

    nc = bacc.Bacc("TRN2", target_bir_lowering=False, debug=False,
                   num_devices=NC, num_swdge_queues=4)

    xT = nc.dram_tensor("xT", (D, N), bf16, kind="ExternalInput")
    xTs = nc.dram_tensor("xTs", (D, NW * 128), bf16, kind="ExternalInput")
    W1T = nc.dram_tensor("W1T", (D, D), bf16, kind="ExternalInput")
    WC = nc.dram_tensor("WC", (D, 260), bf16, kind="ExternalInput")
    CD = nc.dram_tensor("CD", (D, 4), bf16, kind="ExternalInput")
    prelu = nc.dram_tensor("prelu", (D, 1), f32, kind="ExternalInput")
    GIDX = nc.dram_tensor("GIDX", (128, NCALLS * 64), i16, kind="ExternalInput")
    INDB = nc.dram_tensor("INDB", (128, NB * 128), fp8, kind="ExternalInput")
    INDTB = nc.dram_tensor("INDTB", (128, NB * 128), fp8, kind="ExternalInput")
    bias128 = nc.dram_tensor("bias128", (128, D), f32, kind="ExternalInput")
    ones_col = nc.dram_tensor("ones_col", (128, 1), f32, kind="ExternalInput")
    rmask_col = nc.dram_tensor("rmask_col", (128, 1), f32, kind="ExternalInput")
    onesrow = nc.dram_tensor("onesrow", (1, 128), f32, kind="ExternalInput")
    gb_row = nc.dram_tensor("gb_row", (1, 128), f32, kind="ExternalInput")  # [gamma|beta]
    out_slab = nc.dram_tensor("out_slab", (SLAB, D), f32, kind="ExternalOutput")

    with TileContext(nc) as tc:
        with tc.tile_pool(name="dram", bufs=1, space="DRAM") as dpool, \
             tc.tile_pool(name="persist", bufs=1) as pp:
            table = dpool.tile([TROWS, RE], bf16)
            cc_in = dpool.tile([1, 128], f32)
            cc_out = dpool.tile([1, 128], f32)

            w1t_sb = pp.tile([D, D], bf16)
            nc.sync.dma_start(w1t_sb[:], W1T[:, :])
            wc_sb = pp.tile([D, 260], bf16)
            nc.sync.dma_start(wc_sb[:], WC[:, :])
            cd_sb = pp.tile([D, 4], bf16)
            nc.sync.dma_start(cd_sb[:], CD[:, :])
            prelu_sb = pp.tile([D, 1], f32)
            nc.sync.dma_start(prelu_sb[:], prelu[:, :])
            bias_sb = pp.tile([128, D], f32)
            nc.sync.dma_start(bias_sb[:], bias128[:, :])
            ones_sb = pp.tile([128, 1], f32)
            nc.sync.dma_start(ones_sb[:], ones_col[:, :])
            rmask_sb = pp.tile([128, 1], f32)
            nc.sync.dma_start(rmask_sb[:], rmask_col[:, :])
            onesrow_sb = pp.tile([1, 128], f32)
            nc.sync.dma_start(onesrow_sb[:], onesrow[:, :])
            gb_sb = pp.tile([1, 128], f32)
            nc.sync.dma_start(gb_sb[:], gb_row[:, :])
            gidx_sb = pp.tile([128, NCALLS * 64], i16)
            nc.sync.dma_start(gidx_sb[:], GIDX[:, :])

            a_dst = pp.tile([128, NW, 4], bf16)
            slab = pp.tile([128, NW, 260], f32)
            y_sb = pp.tile([128, NW, D], f32)

            # ---------------- phase T: node table + phase A: a_dst ----------
            with tc.tile_pool(name="pt_sb", bufs=3) as tp, \
                 tc.tile_pool(name="pt_ps", bufs=2, space="PSUM") as tps, \
                 tc.tile_pool(name="pt_ps2", bufs=2, space="PSUM") as tps2:
                # pad rows
                padrow = tp.tile([1, RE], bf16, tag="pad")
                nc.vector.memset(padrow[:], 0.0)
                nc.vector.memset(padrow[:].bitcast(f32)[:, 128:132], -1e4)
                nc.sync.dma_start(table[PAD_LO:PAD_LO + 1, :], padrow[:])
                nc.sync.dma_start(table[2 * LO + 1:2 * LO + 2, :], padrow[:])

                n_tiles = (N + 511) // 512
                for t in range(n_tiles):
                    c0 = t * 512
                    nt = min(512, N - c0)
                    xt = tp.tile([D, 512], bf16, tag="xt")
                    nc.sync.dma_start(xt[:, :nt], xT[:, c0:c0 + nt])
                    m1 = tps.tile([D, 512], f32, tag="m1")
                    nc.tensor.matmul(out=m1[:, :nt], lhsT=w1t_sb[:], rhs=xt[:, :nt],
                                     start=True, stop=True)
                    x1w = tp.tile([D, 512], f32, tag="x1w")
                    nc.scalar.mul(x1w[:, :nt], m1[:, :nt], prelu_sb[:, :])
                    x1 = tp.tile([D, 512], bf16, tag="x1")
                    nc.vector.tensor_tensor(out=x1[:, :nt], in0=x1w[:, :nt],
                                            in1=m1[:, :nt], op=AL.max)
                    j = 0
                    while j * 128 < nt:
                        mj = min(128, nt - j * 128)
                        p2 = tps2.tile([128, 260], f32, tag="p2")
                        nc.tensor.matmul(out=p2[:mj, :], lhsT=x1[:, j * 128:j * 128 + mj],
                                         rhs=wc_sb[:], start=True, stop=True)
                        row = tp.tile([128, RE], bf16, tag="row")
                        # table row index offset: rows >= LO shift by 1 (pad row)
                        r0 = c0 + j * 128
                        if (t * 4 + j) % 2 == 0:
                            nc.vector.tensor_copy(row[:mj, :256], p2[:mj, :256])
                        else:
                            nc.scalar.copy(row[:mj, :256], p2[:mj, :256])
                        nc.vector.tensor_copy(row[:mj].bitcast(f32)[:, 128:132],
                                              p2[:mj, 256:260])
                        if r0 + mj <= LO:
                            nc.sync.dma_start(table[r0:r0 + mj, :], row[:mj, :])
                        elif r0 >= LO:
                            nc.sync.dma_start(table[r0 + 1:r0 + 1 + mj, :], row[:mj, :])
                        else:
                            cut = LO - r0
                            nc.sync.dma_start(table[r0:LO, :], row[:cut, :])
                            nc.sync.dma_start(table[LO + 1:LO + 1 + mj - cut, :],
                                              row[cut:mj, :])
                        j += 1

                # phase A: a_dst for own slab (from xTs, padded to NW*128)
                for t in range((NW * 128 + 511) // 512):
                    c0 = t * 512
                    nt = min(512, NW * 128 - c0)
                    xt = tp.tile([D, 512], bf16, tag="xt")
                    nc.sync.dma_start(xt[:, :nt], xTs[:, c0:c0 + nt])
                    m1 = tps.tile([D, 512], f32, tag="m1")
                    nc.tensor.matmul(out=m1[:, :nt], lhsT=w1t_sb[:], rhs=xt[:, :nt],
                                     start=True, stop=True)
                    x1w = tp.tile([D, 512], f32, tag="x1w")
                    nc.scalar.mul(x1w[:, :nt], m1[:, :nt], prelu_sb[:, :])
                    x1 = tp.tile([D, 512], bf16, tag="x1")
                    nc.vector.tensor_tensor(out=x1[:, :nt], in0=x1w[:, :nt],
                                            in1=m1[:, :nt], op=AL.max)
                    j = 0
                    while j * 128 < nt:
                        wdx = (c0 + j * 128) // 128
                        ap2 = tps2.tile([128, 260], f32, tag="p2")
                        nc.tensor.matmul(out=ap2[:, :4], lhsT=x1[:, j * 128:(j + 1) * 128],
                                         rhs=cd_sb[:], start=True, stop=True)
                        nc.vector.tensor_copy(a_dst[:, wdx, :], ap2[:, :4])
                        j += 1

            tc.strict_bb_all_engine_barrier()

            # ---------------- phase E: edges ------------------------------
            with tc.tile_pool(name="pe_g", bufs=3) as gp, \
                 tc.tile_pool(name="pe_i", bufs=3) as ip, \
                 tc.tile_pool(name="pe_s", bufs=3) as sp, \
                 tc.tile_pool(name="pe_wp", bufs=2, space="PSUM") as wp, \
                 tc.tile_pool(name="pe_ap", bufs=2, space="PSUM") as app:
                wpt_by_sec = {}
                for ci in range(NCALLS):
                    kind = sched["call_kind"][ci]
                    tbl = table[0:LO + 1, :] if kind == 0 else table[LO + 1:2 * LO + 2, :]
                    gt = gp.tile([128, BPC, RE], bf16, tag="g")
                    nc.gpsimd.dma_gather(
                        out_ap=gt[:], in_ap=tbl,
                        idxs_ap=gidx_sb[:, ci * 64:(ci + 1) * 64],
                        num_idxs=CALL, num_idxs_reg=CALL, elem_size=RE,
                        queue_num=ci % 4)
                    ind_t = ip.tile([128, BPC * 128], fp8, tag="ind")
                    nc.sync.dma_start(ind_t[:], INDB[:, ci * 1024:(ci + 1) * 1024])
                    indt_t = ip.tile([128, BPC * 128], fp8, tag="indt")
                    nc.sync.dma_start(indt_t[:], INDTB[:, ci * 1024:(ci + 1) * 1024])

                    adst_pt = app.tile([128, BPC, 4], f32, tag="adst")
                    live = []
                    for b in range(BPC):
                        info = binfo[ci * BPC + b]
                        if info["dead"]:
                            continue
                        live.append((b, info))
                        nc.tensor.matmul(
                            out=adst_pt[:, b, :],
                            lhsT=indt_t[:, b * 128:(b + 1) * 128],
                            rhs=a_dst[:, info["w"], :],
                            start=True, stop=True)
                    if not live:
                        continue
                    e0 = sp.tile([128, BPC, 16], f32, tag="e0")
                    nc.vector.tensor_tensor(
                        out=e0[:, :, :4], in0=gt[:].bitcast(f32)[:, :, 128:132],
                        in1=adst_pt[:], op=AL.add)
                    e1 = sp.tile([128, BPC, 16], f32, tag="e1")
                    nc.vector.scalar_tensor_tensor(
                        out=e1[:], in0=e0[:], scalar=NEG, in1=e0[:],
                        op0=AL.mult, op1=AL.max)
                    p_t = sp.tile([128, BPC, 16], f32, tag="p")
                    nc.scalar.activation(p_t[:].rearrange("p a b -> p (a b)"),
                                         e1[:].rearrange("p a b -> p (a b)"), AF.Exp)
                    pb = sp.tile([128, BPC, 16], bf16, tag="pb")
                    nc.vector.tensor_copy(pb[:], p_t[:])
                    msg = sp.tile([128, BPC, 272], bf16, tag="msg")
                    nc.vector.tensor_tensor(
                        out=msg[:, :, :256].rearrange("p c (h d) -> p c h d", h=4),
                        in0=gt[:, :, :256].rearrange("p c (h d) -> p c h d", h=4),
                        in1=pb[:, :, :4].broadcast_to([128, BPC, 4, 64]),
                        op=AL.mult)
                    nc.vector.tensor_copy(msg[:, :, 256:272], pb[:])
                    for b, info in live:
                        s = info["sec"]
                        if info["start"]:
                            wpt_by_sec[s] = wp.tile([128, 260], f32, tag="wpt", name=f"wpt{s}")
                        nc.tensor.matmul(
                            out=wpt_by_sec[s][:],
                            lhsT=ind_t[:, b * 128:(b + 1) * 128],
                            rhs=msg[:, b, :260],
                            start=info["start"], stop=info["stop"])
                        if info["stop"]:
                            wdx = info["w"]
                            if info["kind"] == 0:
                                nc.scalar.copy(slab[:, wdx, :], wpt_by_sec[s][:])
                            else:
                                nc.vector.tensor_tensor(
                                    out=slab[:, wdx, :], in0=slab[:, wdx, :],
                                    in1=wpt_by_sec[s][:], op=AL.add)
                            del wpt_by_sec[s]

            # ---------------- phase F: finalize + BN partials --------------
            with tc.tile_pool(name="pf", bufs=3) as fp_, \
                 tc.tile_pool(name="pf_ps", bufs=1, space="PSUM") as fps, \
                 tc.tile_pool(name="pb_ps", bufs=1, space="PSUM") as bps:
                bn_s = fps.tile([1, D], f32, tag="bns")
                bn_q = fps.tile([1, D], f32, tag="bnq")
                for wdx in range(NW):
                    dn = fp_.tile([128, 4], f32, tag="dn")
                    nc.vector.tensor_scalar_add(dn[:], slab[:, wdx, 256:260], 1e-30)
                    rd = fp_.tile([128, 4], f32, tag="rd")
                    nc.vector.reciprocal(rd[:], dn[:])
                    tt = fp_.tile([128, 256], f32, tag="tt")
                    nc.vector.tensor_tensor(
                        out=tt[:].rearrange("p (h d) -> p h d", h=4),
                        in0=slab[:, wdx, :256].rearrange("p (h d) -> p h d", h=4),
                        in1=rd[:].broadcast_to([128, 4, 64]),
                        op=AL.mult)
                    t2 = fp_.tile([128, 128], f32, tag="t2")
                    nc.vector.tensor_tensor(out=t2[:], in0=tt[:, :128], in1=tt[:, 128:],
                                            op=AL.add)
                    y1 = fp_.tile([128, D], f32, tag="y1")
                    nc.vector.tensor_tensor(out=y1[:], in0=t2[:, :64], in1=t2[:, 64:],
                                            op=AL.add)
                    nc.vector.scalar_tensor_tensor(
                        out=y_sb[:, wdx, :], in0=y1[:], scalar=0.25, in1=bias_sb[:],
                        op0=AL.mult, op1=AL.add)
                    sq = fp_.tile([128, D], f32, tag="sq")
                    nc.scalar.square(sq[:], y_sb[:, wdx, :])
                    msk = ones_sb if wdx < NW - 1 else rmask_sb
                    nc.tensor.matmul(out=bn_s[:], lhsT=msk[:], rhs=y_sb[:, wdx, :],
                                     start=(wdx == 0), stop=(wdx == NW - 1))
                    nc.tensor.matmul(out=bn_q[:], lhsT=msk[:], rhs=sq[:],
                                     start=(wdx == 0), stop=(wdx == NW - 1))

                # ---------------- phase B: BN + relu + store ---------------
                st = fp_.tile([1, 128], f32, tag="st")
                nc.vector.tensor_copy(st[:, :64], bn_s[:])
                nc.vector.tensor_copy(st[:, 64:], bn_q[:])
                nc.gpsimd.dma_start(cc_in[:], st[:])
                nc.gpsimd.collective_compute(
                    "AllReduce", AL.add, replica_groups=[list(range(NC))],
                    ins=[cc_in[:].opt()], outs=[cc_out[:].opt()])
                st2 = fp_.tile([1, 128], f32, tag="st2")
                nc.gpsimd.dma_start(st2[:], cc_out[:])
                mean = fp_.tile([1, D], f32, tag="mean")
                nc.vector.tensor_scalar_mul(mean[:], st2[:, :64], 1.0 / N)
                ex2 = fp_.tile([1, D], f32, tag="ex2")
                nc.vector.tensor_scalar_mul(ex2[:], st2[:, 64:], 1.0 / N)
                msq = fp_.tile([1, D], f32, tag="msq")
                nc.scalar.square(msq[:], mean[:])
                var = fp_.tile([1, D], f32, tag="var")
                nc.vector.tensor_tensor(out=var[:], in0=ex2[:], in1=msq[:],
                                        op=AL.subtract)
                veps = fp_.tile([1, D], f32, tag="veps")
                nc.vector.tensor_scalar_add(veps[:], var[:], BN_EPS)
                sd = fp_.tile([1, D], f32, tag="sd")
                nc.scalar.sqrt(sd[:], veps[:])
                rs = fp_.tile([1, D], f32, tag="rs")
                nc.vector.reciprocal(rs[:], sd[:])
                scsh = fp_.tile([1, 128], f32, tag="scsh")
                nc.vector.tensor_tensor(out=scsh[:, :64], in0=gb_sb[:, :64], in1=rs[:],
                                        op=AL.mult)
                mssc = fp_.tile([1, D], f32, tag="mssc")
                nc.vector.tensor_tensor(out=mssc[:], in0=mean[:], in1=scsh[:, :64],
                                        op=AL.mult)
                nc.vector.tensor_tensor(out=scsh[:, 64:], in0=gb_sb[:, 64:], in1=mssc[:],
                                        op=AL.subtract)
                bc = bps.tile([128, 128], f32, tag="bc")
                nc.tensor.matmul(out=bc[:], lhsT=onesrow_sb[:], rhs=scsh[:],
                                 start=True, stop=True)
                for wdx in range(NW):
                    z = fp_.tile([128, D], f32, tag="z")
                    nc.vector.tensor_tensor(out=z[:], in0=y_sb[:, wdx, :],
                                            in1=bc[:, :64], op=AL.mult)
                    z2 = fp_.tile([128, D], f32, tag="z2")
                    nc.vector.tensor_tensor(out=z2[:], in0=z[:], in1=bc[:, 64:],
                                            op=AL.add)
                    zo = fp_.tile([128, D], f32, tag="zo")
                    nc.scalar.activation(zo[:], z2[:], AF.Relu)
                    rows = W if wdx < NW - 1 else LAST_ROWS
                    nc.sync.dma_start(out_slab[wdx * W:wdx * W + rows, :], zo[:rows, :])

    nc.compile()
    return nc


def kernel(x, edge_index, W_lin, b_lin, prelu_w, W_gat, att_src, att_dst,
           gat_bias, bn_gamma, bn_beta):
    global LAST_EXEC_NS, LAST_TRACE
    from concourse import bass_utils

    x = np.asarray(x, np.float32)
    edge_index = np.asarray(edge_index)
    W_lin = np.asarray(W_lin, np.float32)
    b_lin = np.asarray(b_lin, np.float32)
    prelu_w = np.asarray(prelu_w, np.float32)
    W_gat = np.asarray(W_gat, np.float32)
    att_src = np.asarray(att_src, np.float32)
    att_dst = np.asarray(att_dst, np.float32)
    gat_bias = np.asarray(gat_bias, np.float32)
    bn_gamma = np.asarray(bn_gamma, np.float32)
    bn_beta = np.asarray(bn_beta, np.float32)

    key = hashlib.sha1(np.ascontiguousarray(edge_index).tobytes()).hexdigest()
    if key not in _CACHE:
        sched, blobs = _schedule_and_blobs(edge_index)
        nc = _build_program(sched)
        _CACHE[key] = (sched, blobs, nc)
    sched, blobs, nc = _CACHE[key]

    # b_lin is zero in the reference setup; if nonzero, do the pre-linear
    # exactly on host and feed the device an identity pre-stage.
    if np.any(b_lin != 0):
        x1_host = x @ W_lin.T + b_lin
        x1_host = np.where(x1_host >= 0, x1_host, prelu_w * x1_host)
        # then device treats W_lin as identity and prelu as identity:
        xT_eff = np.ascontiguousarray(x1_host.T)
        W1_eff = np.eye(64, dtype=np.float32)
        prelu_eff = np.ones((64,), np.float32)
    else:
        xT_eff = np.ascontiguousarray(x.T)
        W1_eff = W_lin
        prelu_eff = prelu_w

    C_src = np.zeros((64, 4), np.float32)
    C_dst = np.zeros((64, 4), np.float32)
    for h in range(H):
        Wh = W_gat[h * 64:(h + 1) * 64, :]  # [64, 64] maps x1 -> head h
        C_src[:, h] = Wh.T @ att_src[h]
        C_dst[:, h] = Wh.T @ att_dst[h]

    bf = ml_dtypes.bfloat16
    W1T_np = np.ascontiguousarray(W1_eff.T).astype(bf)  # [din, dout]
    WC_np = np.concatenate([np.ascontiguousarray(W_gat.T), C_src], axis=1).astype(bf)
    CD_np = C_dst.astype(bf)
    xT_bf = xT_eff.astype(bf)

    rmask = np.zeros((128, 1), np.float32)
    rmask[:LAST_ROWS] = 1.0

    in_maps = []
    for c in range(NC):
        xs = np.zeros((64, NW * 128), np.float32)
        xs[:, :SLAB] = xT_eff[:, c * SLAB:(c + 1) * SLAB]
        in_maps.append(dict(
            xT=xT_bf,
            xTs=xs.astype(bf),
            W1T=W1T_np, WC=WC_np, CD=CD_np,
            prelu=prelu_eff.reshape(64, 1),
            GIDX=blobs[c]["GIDX"], INDB=blobs[c]["INDB"], INDTB=blobs[c]["INDTB"],
            bias128=np.tile(gat_bias[None, :], (128, 1)),
            ones_col=np.ones((128, 1), np.float32),
            rmask_col=rmask,
            onesrow=np.ones((1, 128), np.float32),
            gb_row=np.concatenate([bn_gamma, bn_beta])[None, :],
        ))

    trace = os.environ.get("GAT_TRACE", "0") == "1"
    if trace:
        _install_ntff_shim()
    res = bass_utils.run_bass_kernel_spmd(nc, in_maps, core_ids=list(range(NC)),
                                          trace=trace)
    LAST_EXEC_NS = res.exec_time_ns
    LAST_TRACE = res.instructions_and_trace
    out = np.empty((N, D), np.float32)
    for c in range(NC):
        out[c * SLAB:(c + 1) * SLAB] = res.results[c]["out_slab"]
    return out


# revision 10
# speedup vs baseline: 1.0486x; 1.0486x over previous
"""GAT-mod forward on 8 trn2 NeuronCores (Bass/Tile).

Strategy (dst-sharded, slot-major message passing):
- Nodes are partitioned across 8 cores by destination id (6250 each).
- Each core builds the full node table T[n] = [h(n) bf16(256) | a_src(n) f32(4) | pad]
  (768B rows) in its local HBM (recompute is cheaper than all-gather), split
  logically at row 25000 so gather indices fit int16 (dma_gather limit), with a
  PAD row per half (h=0, a_src=-1e4 -> exp underflows to exactly 0).
- Edges (incl. self-loops) are grouped by 128-node destination windows, split
  into lo/hi source halves, packed into 128-slot batches (slot = edge).
  Per batch the host emits fp8 indicator matrices Ind[slot, node] and its
  transpose; the device then:
    gather rows -> e = lrelu(a_src + IndT@a_dst) -> p = exp(e) (no-max softmax;
    e is bounded by construction so exp cannot overflow) -> msg = [g*p | p]
    -> PSUM[node, 260] += Ind^T-weighted sum via PE matmul.
  alpha normalization (p/denom) is applied after aggregation per node.
- BN batch stats via partial sums + AllReduce across the 8 cores.
"""

import os
import sys
import hashlib

import numpy as np
import ml_dtypes

N = 50000
E = 800000
D = 64
H = 4
HD = 256
NEG = 0.2
BN_EPS = 1e-5
NC = 8
SLAB = N // NC          # 6250
W = 128                 # window nodes
NW = (SLAB + W - 1) // W  # 49
LAST_ROWS = SLAB - (NW - 1) * W  # 106
LO = 25000
RE = 384                # table row elems (bf16): 256 h + 8 (4 f32 a_src) + pad
TROWS = 2 * LO + 2      # 50002 (two pad rows)
PAD_LO = LO             # pad row index within lo half
PAD_HI = LO             # within hi half (row 25001+25000 = 50001)
BPC = 8                 # batches per gather call
CALL = BPC * 128        # 1024 idxs per gather

_CACHE = {}
LAST_EXEC_NS = None
LAST_TRACE = None


def _install_ntff_shim():
    import contextlib
    import ctypes
    import types

    if "antenv.axon_hooks" in sys.modules:
        return
    so_path = "/opt/axon/libaxon_pjrt.so"

    def _hook_factory(so_path):
        try:
            lib = ctypes.CDLL(so_path)
        except OSError:
            return None
        if not hasattr(lib, "axon_start_nrt_profile"):
            return None
        lib.axon_start_nrt_profile.argtypes = [ctypes.POINTER(ctypes.c_int64), ctypes.c_size_t]
        lib.axon_start_nrt_profile.restype = ctypes.c_int64
        lib.axon_stop_nrt_profile.argtypes = [ctypes.c_char_p]
        lib.axon_stop_nrt_profile.restype = ctypes.c_int64

        @contextlib.contextmanager
        def _hook(output_dir, device_ids):
            import jax

            jax.devices()
            if device_ids:
                ids = (ctypes.c_int64 * len(device_ids))(*device_ids)
                rc = lib.axon_start_nrt_profile(ids, len(device_ids))
            else:
                rc = lib.axon_start_nrt_profile(None, 0)
            if rc != 0:
                raise RuntimeError(f"axon_start_nrt_profile rc={rc}")
            try:
                yield
            finally:
                lib.axon_stop_nrt_profile(str(output_dir).encode())

        return _hook

    mod = types.ModuleType("antenv.axon_hooks")
    _h = [None]
    mod.set_axon_ntff_profile_hook = lambda h: _h.__setitem__(0, h)
    mod.get_axon_ntff_profile_hook = lambda: _h[0]
    sys.modules["antenv.axon_hooks"] = mod
    try:
        import antenv

        antenv.axon_hooks = mod
    except ImportError:
        pass
    mod.set_axon_ntff_profile_hook(_hook_factory(so_path))


# ----------------------------------------------------------------- host prep
def _schedule_and_blobs(edge_index):
    src = np.concatenate([edge_index[0].astype(np.int64), np.arange(N, dtype=np.int64)])
    dst = np.concatenate([edge_index[1].astype(np.int64), np.arange(N, dtype=np.int64)])

    cores = []
    for c in range(NC):
        sel = (dst >= c * SLAB) & (dst < (c + 1) * SLAB)
        s_src = src[sel]
        s_dst = dst[sel] - c * SLAB
        islo = s_src < LO
        win = s_dst >> 7
        secid = win * 2 + (1 - islo.astype(np.int64))  # even = lo, odd = hi
        order = np.argsort(secid, kind="stable")
        cores.append((s_src[order], s_dst[order], secid[order]))

    # per-(core, section) counts; shared schedule = max over cores
    NSEC = NW * 2
    cnts = np.zeros((NC, NSEC), np.int64)
    for c in range(NC):
        binc = np.bincount(cores[c][2], minlength=NSEC)
        cnts[c] = binc
    nb_sec = (np.max(cnts, axis=0) + 127) // 128  # batches per section
    nb_sec = np.maximum(nb_sec, 1)

    # batch list: lo run (even sections, w ascending), then hi run
    batches = []  # (w, kind, sec, dead)
    for kind in (0, 1):  # 0=lo, 1=hi
        run_start = len(batches)
        for wdx in range(NW):
            s = wdx * 2 + kind
            for _ in range(int(nb_sec[s])):
                batches.append([wdx, kind, s, False])
        while (len(batches) - run_start) % BPC != 0:
            batches.append([0, kind, -1, True])
    NB = len(batches)
    NCALLS = NB // BPC
    call_kind = [batches[ci * BPC][1] for ci in range(NCALLS)]

    # mark section start/stop per batch
    sec_first = {}
    sec_last = {}
    for bi, (wdx, kind, s, dead) in enumerate(batches):
        if dead:
            continue
        if s not in sec_first:
            sec_first[s] = bi
        sec_last[s] = bi
    binfo = []
    for bi, (wdx, kind, s, dead) in enumerate(batches):
        binfo.append(dict(w=wdx, kind=kind, sec=s, dead=dead,
                          start=(not dead and sec_first[s] == bi),
                          stop=(not dead and sec_last[s] == bi)))

    sched = dict(NB=NB, NCALLS=NCALLS, call_kind=call_kind, binfo=binfo)

    # per-core blobs
    blobs = []
    for c in range(NC):
        s_src, s_dst, s_sec = cores[c]
        gidx = np.full((NB * 128,), PAD_LO, np.int64)
        ind = np.zeros((NB, 128, 128), np.float32)
        indt = np.zeros((NB, 128, 128), np.float32)
        # slot assignment: per section, edges fill batches in order
        sec_edge_start = np.zeros(NSEC + 1, np.int64)
        np.cumsum(np.bincount(s_sec, minlength=NSEC), out=sec_edge_start[1:])
        # batch index of each section's first batch
        sec_b0 = {}
        for bi, info in enumerate(binfo):
            if not info["dead"] and info["sec"] not in sec_b0:
                sec_b0[info["sec"]] = bi
        for s in range(NSEC):
            e0, e1 = sec_edge_start[s], sec_edge_start[s + 1]
            if e1 == e0:
                continue
            n = e1 - e0
            b0 = sec_b0[s]
            slots = b0 * 128 + np.arange(n)
            srcs = s_src[e0:e1]
            kind = s & 1
            idxv = np.where(kind == 0, srcs, srcs - LO)
            gidx[slots] = idxv
            node_in_w = (s_dst[e0:e1] - (s >> 1) * 128).astype(np.int64)
            bloc = slots // 128
            sloc = slots % 128
            ind[bloc, sloc, node_in_w] = 1.0
            indt[bloc, node_in_w, sloc] = 1.0
        # wrap gather indices: call ci covers positions [ci*1024, +1024)
        g16 = gidx.astype(np.int16).reshape(NCALLS, 64, 16)
        gw = np.transpose(g16, (0, 2, 1)).reshape(NCALLS, 16, 64)
        gw = np.tile(gw, (1, 8, 1))  # [NCALLS, 128, 64]
        GIDX = np.ascontiguousarray(np.transpose(gw, (1, 0, 2)).reshape(128, NCALLS * 64))
        ncalls = NB // BPC
        both = np.concatenate([ind.reshape(ncalls, BPC, 128, 128),
                               indt.reshape(ncalls, BPC, 128, 128)], axis=1)
        INDB = np.ascontiguousarray(
            np.transpose(both, (2, 0, 1, 3)).reshape(128, NB * 256)).astype(ml_dtypes.float8_e4m3)
        blobs.append(dict(GIDX=GIDX, INDB=INDB))
    return sched, blobs


def _build_program(sched):
    from concourse import bacc, mybir
    from concourse.tile import TileContext

    AL = mybir.AluOpType
    AF = mybir.ActivationFunctionType
    f32 = mybir.dt.float32
    bf16 = mybir.dt.bfloat16
    fp8 = mybir.dt.float8e4
    i16 = mybir.dt.int16

    NB = sched["NB"]
    NCALLS = sched["NCALLS"]
    binfo = sched["binfo"]

    nc = bacc.Bacc("TRN2", target_bir_lowering=False, debug=False,
                   num_devices=NC, num_swdge_queues=4)

    xT = nc.dram_tensor("xT", (D, N), bf16, kind="ExternalInput")
    xTs = nc.dram_tensor("xTs", (D, NW * 128), bf16, kind="ExternalInput")
    W1T = nc.dram_tensor("W1T", (D, D), bf16, kind="ExternalInput")
    WC = nc.dram_tensor("WC", (D, 260), bf16, kind="ExternalInput")
    CD = nc.dram_tensor("CD", (D, 4), bf16, kind="ExternalInput")
    prelu = nc.dram_tensor("prelu", (D, 1), f32, kind="ExternalInput")
    GIDX = nc.dram_tensor("GIDX", (128, NCALLS * 64), i16, kind="ExternalInput")
    INDB = nc.dram_tensor("INDB", (128, NB * 256), fp8, kind="ExternalInput")
    bias128 = nc.dram_tensor("bias128", (128, D), f32, kind="ExternalInput")
    ones_col = nc.dram_tensor("ones_col", (128, 1), f32, kind="ExternalInput")
    rmask_col = nc.dram_tensor("rmask_col", (128, 1), f32, kind="ExternalInput")
    onesrow = nc.dram_tensor("onesrow", (1, 128), f32, kind="ExternalInput")
    gb_row = nc.dram_tensor("gb_row", (1, 128), f32, kind="ExternalInput")  # [gamma|beta]
    out_slab = nc.dram_tensor("out_slab", (SLAB, D), f32, kind="ExternalOutput")

    with TileContext(nc) as tc:
        with tc.tile_pool(name="dram", bufs=1, space="DRAM") as dpool, \
             tc.tile_pool(name="persist", bufs=1) as pp:
            table = dpool.tile([TROWS, RE], bf16)
            cc_in = dpool.tile([1, 128], f32)
            cc_out = dpool.tile([1, 128], f32)

            w1t_sb = pp.tile([D, D], bf16)
            nc.sync.dma_start(w1t_sb[:], W1T[:, :])
            wc_sb = pp.tile([D, 260], bf16)
            nc.sync.dma_start(wc_sb[:], WC[:, :])
            cd_sb = pp.tile([D, 4], bf16)
            nc.sync.dma_start(cd_sb[:], CD[:, :])
            prelu_sb = pp.tile([D, 1], f32)
            nc.sync.dma_start(prelu_sb[:], prelu[:, :])
            bias_sb = pp.tile([128, D], f32)
            nc.sync.dma_start(bias_sb[:], bias128[:, :])
            ones_sb = pp.tile([128, 1], f32)
            nc.sync.dma_start(ones_sb[:], ones_col[:, :])
            rmask_sb = pp.tile([128, 1], f32)
            nc.sync.dma_start(rmask_sb[:], rmask_col[:, :])
            onesrow_sb = pp.tile([1, 128], f32)
            nc.sync.dma_start(onesrow_sb[:], onesrow[:, :])
            gb_sb = pp.tile([1, 128], f32)
            nc.sync.dma_start(gb_sb[:], gb_row[:, :])
            gidx_sb = pp.tile([128, NCALLS * 64], i16)
            nc.sync.dma_start(gidx_sb[:], GIDX[:, :])

            a_dst = pp.tile([128, NW, 4], bf16)
            slab = pp.tile([128, NW, 260], f32)
            y_sb = pp.tile([128, NW, D], f32)

            # ---------------- phase T: node table + phase A: a_dst ----------
            with tc.tile_pool(name="pt_sb", bufs=3) as tp, \
                 tc.tile_pool(name="pt_ps", bufs=2, space="PSUM") as tps, \
                 tc.tile_pool(name="pt_ps2", bufs=2, space="PSUM") as tps2:
                # pad rows
                padrow = tp.tile([1, RE], bf16, tag="pad")
                nc.vector.memset(padrow[:], 0.0)
                nc.vector.memset(padrow[:].bitcast(f32)[:, 128:132], -1e4)
                nc.sync.dma_start(table[PAD_LO:PAD_LO + 1, :], padrow[:])
                nc.sync.dma_start(table[2 * LO + 1:2 * LO + 2, :], padrow[:])

                n_tiles = (N + 511) // 512
                for t in range(n_tiles):
                    c0 = t * 512
                    nt = min(512, N - c0)
                    xt = tp.tile([D, 512], bf16, tag="xt")
                    nc.sync.dma_start(xt[:, :nt], xT[:, c0:c0 + nt])
                    m1 = tps.tile([D, 512], f32, tag="m1")
                    nc.tensor.matmul(out=m1[:, :nt], lhsT=w1t_sb[:], rhs=xt[:, :nt],
                                     start=True, stop=True)
                    x1w = tp.tile([D, 512], f32, tag="x1w")
                    nc.scalar.mul(x1w[:, :nt], m1[:, :nt], prelu_sb[:, :])
                    x1 = tp.tile([D, 512], bf16, tag="x1")
                    nc.vector.tensor_tensor(out=x1[:, :nt], in0=x1w[:, :nt],
                                            in1=m1[:, :nt], op=AL.max)
                    j = 0
                    while j * 128 < nt:
                        mj = min(128, nt - j * 128)
                        p2 = tps2.tile([128, 260], f32, tag="p2")
                        nc.tensor.matmul(out=p2[:mj, :], lhsT=x1[:, j * 128:j * 128 + mj],
                                         rhs=wc_sb[:], start=True, stop=True)
                        row = tp.tile([128, RE], bf16, tag="row")
                        # table row index offset: rows >= LO shift by 1 (pad row)
                        r0 = c0 + j * 128
                        if (t * 4 + j) % 2 == 0:
                            nc.vector.tensor_copy(row[:mj, :256], p2[:mj, :256])
                        else:
                            nc.scalar.copy(row[:mj, :256], p2[:mj, :256])
                        nc.vector.tensor_copy(row[:mj].bitcast(f32)[:, 128:132],
                                              p2[:mj, 256:260])
                        if r0 + mj <= LO:
                            nc.sync.dma_start(table[r0:r0 + mj, :], row[:mj, :])
                        elif r0 >= LO:
                            nc.sync.dma_start(table[r0 + 1:r0 + 1 + mj, :], row[:mj, :])
                        else:
                            cut = LO - r0
                            nc.sync.dma_start(table[r0:LO, :], row[:cut, :])
                            nc.sync.dma_start(table[LO + 1:LO + 1 + mj - cut, :],
                                              row[cut:mj, :])
                        j += 1

                # phase A: a_dst for own slab (from xTs, padded to NW*128)
                for t in range((NW * 128 + 511) // 512):
                    c0 = t * 512
                    nt = min(512, NW * 128 - c0)
                    xt = tp.tile([D, 512], bf16, tag="xt")
                    nc.sync.dma_start(xt[:, :nt], xTs[:, c0:c0 + nt])
                    m1 = tps.tile([D, 512], f32, tag="m1")
                    nc.tensor.matmul(out=m1[:, :nt], lhsT=w1t_sb[:], rhs=xt[:, :nt],
                                     start=True, stop=True)
                    x1w = tp.tile([D, 512], f32, tag="x1w")
                    nc.scalar.mul(x1w[:, :nt], m1[:, :nt], prelu_sb[:, :])
                    x1 = tp.tile([D, 512], bf16, tag="x1")
                    nc.vector.tensor_tensor(out=x1[:, :nt], in0=x1w[:, :nt],
                                            in1=m1[:, :nt], op=AL.max)
                    j = 0
                    while j * 128 < nt:
                        wdx = (c0 + j * 128) // 128
                        ap2 = tps2.tile([128, 260], f32, tag="p2")
                        nc.tensor.matmul(out=ap2[:, :4], lhsT=x1[:, j * 128:(j + 1) * 128],
                                         rhs=cd_sb[:], start=True, stop=True)
                        nc.vector.tensor_copy(a_dst[:, wdx, :], ap2[:, :4])
                        j += 1

            tc.strict_bb_all_engine_barrier()

            # ---------------- phase E: edges ------------------------------
            with tc.tile_pool(name="pe_g", bufs=3) as gp, \
                 tc.tile_pool(name="pe_i", bufs=3) as ip, \
                 tc.tile_pool(name="pe_s", bufs=3) as sp, \
                 tc.tile_pool(name="pe_wp", bufs=2, space="PSUM") as wp, \
                 tc.tile_pool(name="pe_ap", bufs=2, space="PSUM") as app:
                wpt_by_sec = {}
                dpt_by_sec = {}
                for ci in range(NCALLS):
                    kind = sched["call_kind"][ci]
                    tbl = table[0:LO + 1, :] if kind == 0 else table[LO + 1:2 * LO + 2, :]
                    gt = gp.tile([128, BPC, RE], bf16, tag="g")
                    nc.gpsimd.dma_gather(
                        out_ap=gt[:], in_ap=tbl,
                        idxs_ap=gidx_sb[:, ci * 64:(ci + 1) * 64],
                        num_idxs=CALL, num_idxs_reg=CALL, elem_size=RE,
                        queue_num=ci % 4)
                    indall = ip.tile([128, BPC * 256], fp8, tag="ind")
                    nc.sync.dma_start(indall[:], INDB[:, ci * 2048:(ci + 1) * 2048])
                    ind_t = indall[:, :BPC * 128]
                    indt_t = indall[:, BPC * 128:]

                    adst_pt = app.tile([128, BPC, 4], f32, tag="adst")
                    live = []
                    for b in range(BPC):
                        info = binfo[ci * BPC + b]
                        if info["dead"]:
                            continue
                        live.append((b, info))
                        nc.tensor.matmul(
                            out=adst_pt[:, b, :],
                            lhsT=indt_t[:, b * 128:(b + 1) * 128],
                            rhs=a_dst[:, info["w"], :],
                            start=True, stop=True)
                    if not live:
                        continue
                    e0 = sp.tile([128, BPC, 16], f32, tag="e0")
                    nc.vector.tensor_tensor(
                        out=e0[:, :, :4], in0=gt[:].bitcast(f32)[:, :, 128:132],
                        in1=adst_pt[:], op=AL.add)
                    e1 = sp.tile([128, BPC, 16], f32, tag="e1")
                    nc.vector.scalar_tensor_tensor(
                        out=e1[:], in0=e0[:], scalar=NEG, in1=e0[:],
                        op0=AL.mult, op1=AL.max)
                    pb = sp.tile([128, BPC, 16], bf16, tag="pb")
                    nc.scalar.activation(pb[:].rearrange("p a b -> p (a b)"),
                                         e1[:].rearrange("p a b -> p (a b)"), AF.Exp)
                    msg = sp.tile([128, BPC, 256], bf16, tag="msg")
                    nc.vector.tensor_tensor(
                        out=msg[:].rearrange("p c (h d) -> p c h d", h=4),
                        in0=gt[:, :, :256].rearrange("p c (h d) -> p c h d", h=4),
                        in1=pb[:, :, :4].broadcast_to([128, BPC, 4, 64]),
                        op=AL.mult)
                    for b, info in live:
                        s = info["sec"]
                        if info["start"]:
                            wpt_by_sec[s] = wp.tile([128, 256], f32, tag="wpt", name=f"wpt{s}")
                            dpt_by_sec[s] = app.tile([128, 4], f32, tag="dpt", name=f"dpt{s}")
                        nc.tensor.matmul(
                            out=wpt_by_sec[s][:],
                            lhsT=ind_t[:, b * 128:(b + 1) * 128],
                            rhs=msg[:, b, :],
                            start=info["start"], stop=info["stop"])
                        nc.tensor.matmul(
                            out=dpt_by_sec[s][:],
                            lhsT=ind_t[:, b * 128:(b + 1) * 128],
                            rhs=pb[:, b, :4],
                            start=info["start"], stop=info["stop"])
                        if info["stop"]:
                            wdx = info["w"]
                            if info["kind"] == 0:
                                nc.scalar.copy(slab[:, wdx, :256], wpt_by_sec[s][:])
                                nc.vector.tensor_copy(slab[:, wdx, 256:260], dpt_by_sec[s][:])
                            else:
                                nc.vector.tensor_tensor(
                                    out=slab[:, wdx, :256], in0=slab[:, wdx, :256],
                                    in1=wpt_by_sec[s][:], op=AL.add)
                                nc.vector.tensor_tensor(
                                    out=slab[:, wdx, 256:260], in0=slab[:, wdx, 256:260],
                                    in1=dpt_by_sec[s][:], op=AL.add)
                            del wpt_by_sec[s]
                            del dpt_by_sec[s]

            # ---------------- phase F: finalize + BN partials --------------
            with tc.tile_pool(name="pf", bufs=3) as fp_, \
                 tc.tile_pool(name="pf_ps", bufs=1, space="PSUM") as fps, \
                 tc.tile_pool(name="pb_ps", bufs=1, space="PSUM") as bps:
                bn_s = fps.tile([1, D], f32, tag="bns")
                bn_q = fps.tile([1, D], f32, tag="bnq")
                for wdx in range(NW):
                    dn = fp_.tile([128, 4], f32, tag="dn")
                    nc.vector.tensor_scalar_add(dn[:], slab[:, wdx, 256:260], 1e-30)
                    rd = fp_.tile([128, 4], f32, tag="rd")
                    nc.vector.reciprocal(rd[:], dn[:])
                    tt = fp_.tile([128, 256], f32, tag="tt")
                    nc.vector.tensor_tensor(
                        out=tt[:].rearrange("p (h d) -> p h d", h=4),
                        in0=slab[:, wdx, :256].rearrange("p (h d) -> p h d", h=4),
                        in1=rd[:].broadcast_to([128, 4, 64]),
                        op=AL.mult)
                    t2 = fp_.tile([128, 128], f32, tag="t2")
                    nc.vector.tensor_tensor(out=t2[:], in0=tt[:, :128], in1=tt[:, 128:],
                                            op=AL.add)
                    y1 = fp_.tile([128, D], f32, tag="y1")
                    nc.vector.tensor_tensor(out=y1[:], in0=t2[:, :64], in1=t2[:, 64:],
                                            op=AL.add)
                    nc.vector.scalar_tensor_tensor(
                        out=y_sb[:, wdx, :], in0=y1[:], scalar=0.25, in1=bias_sb[:],
                        op0=AL.mult, op1=AL.add)
                    sq = fp_.tile([128, D], f32, tag="sq")
                    nc.scalar.square(sq[:], y_sb[:, wdx, :])
                    msk = ones_sb if wdx < NW - 1 else rmask_sb
                    nc.tensor.matmul(out=bn_s[:], lhsT=msk[:], rhs=y_sb[:, wdx, :],
                                     start=(wdx == 0), stop=(wdx == NW - 1))
                    nc.tensor.matmul(out=bn_q[:], lhsT=msk[:], rhs=sq[:],
                                     start=(wdx == 0), stop=(wdx == NW - 1))

                # ---------------- phase B: BN + relu + store ---------------
                st = fp_.tile([1, 128], f32, tag="st")
                nc.vector.tensor_copy(st[:, :64], bn_s[:])
                nc.vector.tensor_copy(st[:, 64:], bn_q[:])
                nc.gpsimd.dma_start(cc_in[:], st[:])
                nc.gpsimd.collective_compute(
                    "AllReduce", AL.add, replica_groups=[list(range(NC))],
                    ins=[cc_in[:].opt()], outs=[cc_out[:].opt()])
                st2 = fp_.tile([1, 128], f32, tag="st2")
                nc.gpsimd.dma_start(st2[:], cc_out[:])
                mean = fp_.tile([1, D], f32, tag="mean")
                nc.vector.tensor_scalar_mul(mean[:], st2[:, :64], 1.0 / N)
                ex2 = fp_.tile([1, D], f32, tag="ex2")
                nc.vector.tensor_scalar_mul(ex2[:], st2[:, 64:], 1.0 / N)
                msq = fp_.tile([1, D], f32, tag="msq")
                nc.scalar.square(msq[:], mean[:])
                var = fp_.tile([1, D], f32, tag="var")
                nc.vector.tensor_tensor(out=var[:], in0=ex2[:], in1=msq[:],
                                        op=AL.subtract)
                veps = fp_.tile([1, D], f32, tag="veps")
                nc.vector.tensor_scalar_add(veps[:], var[:], BN_EPS)
                sd = fp_.tile([1, D], f32, tag="sd")
                nc.scalar.sqrt(sd[:], veps[:])
                rs = fp_.tile([1, D], f32, tag="rs")
                nc.vector.reciprocal(rs[:], sd[:])
                scsh = fp_.tile([1, 128], f32, tag="scsh")
                nc.vector.tensor_tensor(out=scsh[:, :64], in0=gb_sb[:, :64], in1=rs[:],
                                        op=AL.mult)
                mssc = fp_.tile([1, D], f32, tag="mssc")
                nc.vector.tensor_tensor(out=mssc[:], in0=mean[:], in1=scsh[:, :64],
                                        op=AL.mult)
                nc.vector.tensor_tensor(out=scsh[:, 64:], in0=gb_sb[:, 64:], in1=mssc[:],
                                        op=AL.subtract)
                bc = bps.tile([128, 128], f32, tag="bc")
                nc.tensor.matmul(out=bc[:], lhsT=onesrow_sb[:], rhs=scsh[:],
                                 start=True, stop=True)
                for wdx in range(NW):
                    z = fp_.tile([128, D], f32, tag="z")
                    nc.vector.tensor_tensor(out=z[:], in0=y_sb[:, wdx, :],
                                            in1=bc[:, :64], op=AL.mult)
                    z2 = fp_.tile([128, D], f32, tag="z2")
                    nc.vector.tensor_tensor(out=z2[:], in0=z[:], in1=bc[:, 64:],
                                            op=AL.add)
                    zo = fp_.tile([128, D], f32, tag="zo")
                    nc.scalar.activation(zo[:], z2[:], AF.Relu)
                    rows = W if wdx < NW - 1 else LAST_ROWS
                    nc.sync.dma_start(out_slab[wdx * W:wdx * W + rows, :], zo[:rows, :])

    nc.compile()
    return nc


def kernel(x, edge_index, W_lin, b_lin, prelu_w, W_gat, att_src, att_dst,
           gat_bias, bn_gamma, bn_beta):
    global LAST_EXEC_NS, LAST_TRACE
    from concourse import bass_utils

    x = np.asarray(x, np.float32)
    edge_index = np.asarray(edge_index)
    W_lin = np.asarray(W_lin, np.float32)
    b_lin = np.asarray(b_lin, np.float32)
    prelu_w = np.asarray(prelu_w, np.float32)
    W_gat = np.asarray(W_gat, np.float32)
    att_src = np.asarray(att_src, np.float32)
    att_dst = np.asarray(att_dst, np.float32)
    gat_bias = np.asarray(gat_bias, np.float32)
    bn_gamma = np.asarray(bn_gamma, np.float32)
    bn_beta = np.asarray(bn_beta, np.float32)

    key = hashlib.sha1(np.ascontiguousarray(edge_index).tobytes()).hexdigest()
    if key not in _CACHE:
        sched, blobs = _schedule_and_blobs(edge_index)
        nc = _build_program(sched)
        _CACHE[key] = (sched, blobs, nc)
    sched, blobs, nc = _CACHE[key]

    # b_lin is zero in the reference setup; if nonzero, do the pre-linear
    # exactly on host and feed the device an identity pre-stage.
    if np.any(b_lin != 0):
        x1_host = x @ W_lin.T + b_lin
        x1_host = np.where(x1_host >= 0, x1_host, prelu_w * x1_host)
        # then device treats W_lin as identity and prelu as identity:
        xT_eff = np.ascontiguousarray(x1_host.T)
        W1_eff = np.eye(64, dtype=np.float32)
        prelu_eff = np.ones((64,), np.float32)
    else:
        xT_eff = np.ascontiguousarray(x.T)
        W1_eff = W_lin
        prelu_eff = prelu_w

    C_src = np.zeros((64, 4), np.float32)
    C_dst = np.zeros((64, 4), np.float32)
    for h in range(H):
        Wh = W_gat[h * 64:(h + 1) * 64, :]  # [64, 64] maps x1 -> head h
        C_src[:, h] = Wh.T @ att_src[h]
        C_dst[:, h] = Wh.T @ att_dst[h]

    bf = ml_dtypes.bfloat16
    W1T_np = np.ascontiguousarray(W1_eff.T).astype(bf)  # [din, dout]
    WC_np = np.concatenate([np.ascontiguousarray(W_gat.T), C_src], axis=1).astype(bf)
    CD_np = C_dst.astype(bf)
    xT_bf = xT_eff.astype(bf)

    rmask = np.zeros((128, 1), np.float32)
    rmask[:LAST_ROWS] = 1.0

    in_maps = []
    for c in range(NC):
        xs = np.zeros((64, NW * 128), np.float32)
        xs[:, :SLAB] = xT_eff[:, c * SLAB:(c + 1) * SLAB]
        in_maps.append(dict(
            xT=xT_bf,
            xTs=xs.astype(bf),
            W1T=W1T_np, WC=WC_np, CD=CD_np,
            prelu=prelu_eff.reshape(64, 1),
            GIDX=blobs[c]["GIDX"], INDB=blobs[c]["INDB"],
            bias128=np.tile(gat_bias[None, :], (128, 1)),
            ones_col=np.ones((128, 1), np.float32),
            rmask_col=rmask,
            onesrow=np.ones((1, 128), np.float32),
            gb_row=np.concatenate([bn_gamma, bn_beta])[None, :],
        ))

    trace = os.environ.get("GAT_TRACE", "0") == "1"
    if trace:
        _install_ntff_shim()
    res = bass_utils.run_bass_kernel_spmd(nc, in_maps, core_ids=list(range(NC)),
                                          trace=trace)
    LAST_EXEC_NS = res.exec_time_ns
    LAST_TRACE = res.instructions_and_trace
    out = np.empty((N, D), np.float32)
    for c in range(NC):
        out[c * SLAB:(c + 1) * SLAB] = res.results[c]["out_slab"]
    return out


# revision 12
# speedup vs baseline: 1.1374x; 1.0847x over previous
"""GAT-mod forward on 8 trn2 NeuronCores (Bass/Tile).

Strategy (dst-sharded, slot-major message passing):
- Nodes are partitioned across 8 cores by destination id (6250 each).
- Each core builds the full node table T[n] = [h(n) bf16(256) | a_src(n) f32(4) | pad]
  (768B rows) in its local HBM (recompute is cheaper than all-gather), split
  logically at row 25000 so gather indices fit int16 (dma_gather limit), with a
  PAD row per half (h=0, a_src=-1e4 -> exp underflows to exactly 0).
- Edges (incl. self-loops) are grouped by 128-node destination windows, split
  into lo/hi source halves, packed into 128-slot batches (slot = edge).
  Per batch the host emits fp8 indicator matrices Ind[slot, node] and its
  transpose; the device then:
    gather rows -> e = lrelu(a_src + IndT@a_dst) -> p = exp(e) (no-max softmax;
    e is bounded by construction so exp cannot overflow) -> msg = [g*p | p]
    -> PSUM[node, 260] += Ind^T-weighted sum via PE matmul.
  alpha normalization (p/denom) is applied after aggregation per node.
- BN batch stats via partial sums + AllReduce across the 8 cores.
"""

import os
import sys
import hashlib

import numpy as np
import ml_dtypes

N = 50000
E = 800000
D = 64
H = 4
HD = 256
NEG = 0.2
BN_EPS = 1e-5
NC = 8
SLAB = N // NC          # 6250
W = 128                 # window nodes
NW = (SLAB + W - 1) // W  # 49
LAST_ROWS = SLAB - (NW - 1) * W  # 106
LO = 25000
RE = 384                # table row elems (bf16): 256 h + 8 (4 f32 a_src) + pad
TROWS = 2 * LO + 2      # 50002 (two pad rows)
PAD_LO = LO             # pad row index within lo half
PAD_HI = LO             # within hi half (row 25001+25000 = 50001)
BPC = 8                 # batches per gather call
CALL = BPC * 128        # 1024 idxs per gather

_CACHE = {}
LAST_EXEC_NS = None
LAST_TRACE = None


def _install_ntff_shim():
    import contextlib
    import ctypes
    import types

    if "antenv.axon_hooks" in sys.modules:
        return
    so_path = "/opt/axon/libaxon_pjrt.so"

    def _hook_factory(so_path):
        try:
            lib = ctypes.CDLL(so_path)
        except OSError:
            return None
        if not hasattr(lib, "axon_start_nrt_profile"):
            return None
        lib.axon_start_nrt_profile.argtypes = [ctypes.POINTER(ctypes.c_int64), ctypes.c_size_t]
        lib.axon_start_nrt_profile.restype = ctypes.c_int64
        lib.axon_stop_nrt_profile.argtypes = [ctypes.c_char_p]
        lib.axon_stop_nrt_profile.restype = ctypes.c_int64

        @contextlib.contextmanager
        def _hook(output_dir, device_ids):
            import jax

            jax.devices()
            if device_ids:
                ids = (ctypes.c_int64 * len(device_ids))(*device_ids)
                rc = lib.axon_start_nrt_profile(ids, len(device_ids))
            else:
                rc = lib.axon_start_nrt_profile(None, 0)
            if rc != 0:
                raise RuntimeError(f"axon_start_nrt_profile rc={rc}")
            try:
                yield
            finally:
                lib.axon_stop_nrt_profile(str(output_dir).encode())

        return _hook

    mod = types.ModuleType("antenv.axon_hooks")
    _h = [None]
    mod.set_axon_ntff_profile_hook = lambda h: _h.__setitem__(0, h)
    mod.get_axon_ntff_profile_hook = lambda: _h[0]
    sys.modules["antenv.axon_hooks"] = mod
    try:
        import antenv

        antenv.axon_hooks = mod
    except ImportError:
        pass
    mod.set_axon_ntff_profile_hook(_hook_factory(so_path))


# ----------------------------------------------------------------- host prep
def _schedule_and_blobs(edge_index):
    src = np.concatenate([edge_index[0].astype(np.int64), np.arange(N, dtype=np.int64)])
    dst = np.concatenate([edge_index[1].astype(np.int64), np.arange(N, dtype=np.int64)])

    cores = []
    for c in range(NC):
        sel = (dst >= c * SLAB) & (dst < (c + 1) * SLAB)
        s_src = src[sel]
        s_dst = dst[sel] - c * SLAB
        islo = s_src < LO
        win = s_dst >> 7
        secid = win * 2 + (1 - islo.astype(np.int64))  # even = lo, odd = hi
        order = np.argsort(secid, kind="stable")
        cores.append((s_src[order], s_dst[order], secid[order]))

    # per-(core, section) counts; shared schedule = max over cores
    NSEC = NW * 2
    cnts = np.zeros((NC, NSEC), np.int64)
    for c in range(NC):
        binc = np.bincount(cores[c][2], minlength=NSEC)
        cnts[c] = binc
    nb_sec = (np.max(cnts, axis=0) + 127) // 128  # batches per section
    nb_sec = np.maximum(nb_sec, 1)

    # batch list: lo run (even sections, w ascending), then hi run
    batches = []  # (w, kind, sec, dead)
    for kind in (0, 1):  # 0=lo, 1=hi
        run_start = len(batches)
        for wdx in range(NW):
            s = wdx * 2 + kind
            for _ in range(int(nb_sec[s])):
                batches.append([wdx, kind, s, False])
        while (len(batches) - run_start) % BPC != 0:
            batches.append([0, kind, -1, True])
    NB = len(batches)
    NCALLS = NB // BPC
    call_kind = [batches[ci * BPC][1] for ci in range(NCALLS)]

    # mark section start/stop per batch
    sec_first = {}
    sec_last = {}
    for bi, (wdx, kind, s, dead) in enumerate(batches):
        if dead:
            continue
        if s not in sec_first:
            sec_first[s] = bi
        sec_last[s] = bi
    binfo = []
    for bi, (wdx, kind, s, dead) in enumerate(batches):
        binfo.append(dict(w=wdx, kind=kind, sec=s, dead=dead,
                          start=(not dead and sec_first[s] == bi),
                          stop=(not dead and sec_last[s] == bi)))

    sched = dict(NB=NB, NCALLS=NCALLS, call_kind=call_kind, binfo=binfo)

    # per-core blobs
    blobs = []
    for c in range(NC):
        s_src, s_dst, s_sec = cores[c]
        gidx = np.full((NB * 128,), PAD_LO, np.int64)
        ind = np.zeros((NB, 128, 128), np.float32)
        indt = np.zeros((NB, 128, 128), np.float32)
        # slot assignment: per section, edges fill batches in order
        sec_edge_start = np.zeros(NSEC + 1, np.int64)
        np.cumsum(np.bincount(s_sec, minlength=NSEC), out=sec_edge_start[1:])
        # batch index of each section's first batch
        sec_b0 = {}
        for bi, info in enumerate(binfo):
            if not info["dead"] and info["sec"] not in sec_b0:
                sec_b0[info["sec"]] = bi
        for s in range(NSEC):
            e0, e1 = sec_edge_start[s], sec_edge_start[s + 1]
            if e1 == e0:
                continue
            n = e1 - e0
            b0 = sec_b0[s]
            slots = b0 * 128 + np.arange(n)
            srcs = s_src[e0:e1]
            kind = s & 1
            idxv = np.where(kind == 0, srcs, srcs - LO)
            gidx[slots] = idxv
            node_in_w = (s_dst[e0:e1] - (s >> 1) * 128).astype(np.int64)
            bloc = slots // 128
            sloc = slots % 128
            ind[bloc, sloc, node_in_w] = 1.0
            indt[bloc, node_in_w, sloc] = 1.0
        # wrap gather indices: call ci covers positions [ci*1024, +1024)
        g16 = gidx.astype(np.int16).reshape(NCALLS, 64, 16)
        gw = np.transpose(g16, (0, 2, 1)).reshape(NCALLS, 16, 64)
        gw = np.tile(gw, (1, 8, 1))  # [NCALLS, 128, 64]
        GIDX = np.ascontiguousarray(np.transpose(gw, (1, 0, 2)).reshape(128, NCALLS * 64))
        ncalls = NB // BPC
        both = np.concatenate([ind.reshape(ncalls, BPC, 128, 128),
                               indt.reshape(ncalls, BPC, 128, 128)], axis=1)
        INDB = np.ascontiguousarray(
            np.transpose(both, (2, 0, 1, 3)).reshape(128, NB * 256)).astype(ml_dtypes.float8_e4m3)
        blobs.append(dict(GIDX=GIDX, INDB=INDB))
    return sched, blobs


def _build_program(sched):
    from concourse import bacc, mybir
    from concourse.tile import TileContext

    AL = mybir.AluOpType
    AF = mybir.ActivationFunctionType
    f32 = mybir.dt.float32
    bf16 = mybir.dt.bfloat16
    fp8 = mybir.dt.float8e4
    i16 = mybir.dt.int16

    NB = sched["NB"]
    NCALLS = sched["NCALLS"]
    binfo = sched["binfo"]

    nc = bacc.Bacc("TRN2", target_bir_lowering=False, debug=False,
                   num_devices=NC, num_swdge_queues=4)

    xT = nc.dram_tensor("xT", (D, N), bf16, kind="ExternalInput")
    xTs = nc.dram_tensor("xTs", (D, NW * 128), bf16, kind="ExternalInput")
    W1T = nc.dram_tensor("W1T", (D, D), bf16, kind="ExternalInput")
    WC = nc.dram_tensor("WC", (D, 260), bf16, kind="ExternalInput")
    CD = nc.dram_tensor("CD", (D, 4), bf16, kind="ExternalInput")
    prelu = nc.dram_tensor("prelu", (D, 1), f32, kind="ExternalInput")
    GIDX = nc.dram_tensor("GIDX", (128, NCALLS * 64), i16, kind="ExternalInput")
    INDB = nc.dram_tensor("INDB", (128, NB * 256), fp8, kind="ExternalInput")
    bias128 = nc.dram_tensor("bias128", (128, D), f32, kind="ExternalInput")
    ones_col = nc.dram_tensor("ones_col", (128, 1), f32, kind="ExternalInput")
    rmask_col = nc.dram_tensor("rmask_col", (128, 1), f32, kind="ExternalInput")
    onesrow = nc.dram_tensor("onesrow", (1, 128), f32, kind="ExternalInput")
    gb_row = nc.dram_tensor("gb_row", (1, 128), f32, kind="ExternalInput")  # [gamma|beta]
    out_slab = nc.dram_tensor("out_slab", (SLAB, D), f32, kind="ExternalOutput")

    with TileContext(nc) as tc:
        with tc.tile_pool(name="dram", bufs=1, space="DRAM") as dpool, \
             tc.tile_pool(name="persist", bufs=1) as pp:
            table = dpool.tile([TROWS, RE], bf16)
            cc_in = dpool.tile([1, 128], f32)
            cc_out = dpool.tile([1, 128], f32)

            w1t_sb = pp.tile([D, D], bf16)
            nc.sync.dma_start(w1t_sb[:], W1T[:, :])
            wc_sb = pp.tile([D, 260], bf16)
            nc.sync.dma_start(wc_sb[:], WC[:, :])
            cd_sb = pp.tile([D, 4], bf16)
            nc.sync.dma_start(cd_sb[:], CD[:, :])
            prelu_sb = pp.tile([D, 1], f32)
            nc.sync.dma_start(prelu_sb[:], prelu[:, :])
            bias_sb = pp.tile([128, D], f32)
            nc.sync.dma_start(bias_sb[:], bias128[:, :])
            ones_sb = pp.tile([128, 1], f32)
            nc.sync.dma_start(ones_sb[:], ones_col[:, :])
            rmask_sb = pp.tile([128, 1], f32)
            nc.sync.dma_start(rmask_sb[:], rmask_col[:, :])
            onesrow_sb = pp.tile([1, 128], f32)
            nc.sync.dma_start(onesrow_sb[:], onesrow[:, :])
            gb_sb = pp.tile([1, 128], f32)
            nc.sync.dma_start(gb_sb[:], gb_row[:, :])
            gidx_sb = pp.tile([128, NCALLS * 64], i16)
            nc.sync.dma_start(gidx_sb[:], GIDX[:, :])

            a_dst = pp.tile([128, NW, 4], bf16)
            slab = pp.tile([128, NW, 260], f32)
            y_sb = pp.tile([128, NW, D], f32)

            # ---------------- phase T: node table + phase A: a_dst ----------
            with tc.tile_pool(name="pt_sb", bufs=3) as tp, \
                 tc.tile_pool(name="pt_ps", bufs=2, space="PSUM") as tps, \
                 tc.tile_pool(name="pt_ps2", bufs=2, space="PSUM") as tps2:
                # pad rows
                padrow = tp.tile([1, RE], bf16, tag="pad")
                nc.vector.memset(padrow[:], 0.0)
                nc.vector.memset(padrow[:].bitcast(f32)[:, 128:132], -1e4)
                nc.sync.dma_start(table[PAD_LO:PAD_LO + 1, :], padrow[:])
                nc.sync.dma_start(table[2 * LO + 1:2 * LO + 2, :], padrow[:])

                n_tiles = (N + 511) // 512
                for t in range(n_tiles):
                    c0 = t * 512
                    nt = min(512, N - c0)
                    xt = tp.tile([D, 512], bf16, tag="xt")
                    nc.sync.dma_start(xt[:, :nt], xT[:, c0:c0 + nt])
                    m1 = tps.tile([D, 512], f32, tag="m1")
                    nc.tensor.matmul(out=m1[:, :nt], lhsT=w1t_sb[:], rhs=xt[:, :nt],
                                     start=True, stop=True)
                    x1w = tp.tile([D, 512], f32, tag="x1w")
                    nc.scalar.mul(x1w[:, :nt], m1[:, :nt], prelu_sb[:, :])
                    x1 = tp.tile([D, 512], bf16, tag="x1")
                    nc.vector.tensor_tensor(out=x1[:, :nt], in0=x1w[:, :nt],
                                            in1=m1[:, :nt], op=AL.max)
                    j = 0
                    while j * 128 < nt:
                        mj = min(128, nt - j * 128)
                        p2 = tps2.tile([128, 260], f32, tag="p2")
                        nc.tensor.matmul(out=p2[:mj, :], lhsT=x1[:, j * 128:j * 128 + mj],
                                         rhs=wc_sb[:], start=True, stop=True)
                        row = tp.tile([128, RE], bf16, tag="row")
                        # table row index offset: rows >= LO shift by 1 (pad row)
                        r0 = c0 + j * 128
                        if (t * 4 + j) % 2 == 0:
                            nc.vector.tensor_copy(row[:mj, :256], p2[:mj, :256])
                        else:
                            nc.scalar.copy(row[:mj, :256], p2[:mj, :256])
                        nc.vector.tensor_copy(row[:mj].bitcast(f32)[:, 128:132],
                                              p2[:mj, 256:260])
                        if r0 + mj <= LO:
                            nc.sync.dma_start(table[r0:r0 + mj, :264], row[:mj, :264])
                        elif r0 >= LO:
                            nc.sync.dma_start(table[r0 + 1:r0 + 1 + mj, :264],
                                              row[:mj, :264])
                        else:
                            cut = LO - r0
                            nc.sync.dma_start(table[r0:LO, :264], row[:cut, :264])
                            nc.sync.dma_start(table[LO + 1:LO + 1 + mj - cut, :264],
                                              row[cut:mj, :264])
                        j += 1

                # phase A: a_dst for own slab (from xTs, padded to NW*128)
                for t in range((NW * 128 + 511) // 512):
                    c0 = t * 512
                    nt = min(512, NW * 128 - c0)
                    xt = tp.tile([D, 512], bf16, tag="xt")
                    nc.sync.dma_start(xt[:, :nt], xTs[:, c0:c0 + nt])
                    m1 = tps.tile([D, 512], f32, tag="m1")
                    nc.tensor.matmul(out=m1[:, :nt], lhsT=w1t_sb[:], rhs=xt[:, :nt],
                                     start=True, stop=True)
                    x1w = tp.tile([D, 512], f32, tag="x1w")
                    nc.scalar.mul(x1w[:, :nt], m1[:, :nt], prelu_sb[:, :])
                    x1 = tp.tile([D, 512], bf16, tag="x1")
                    nc.vector.tensor_tensor(out=x1[:, :nt], in0=x1w[:, :nt],
                                            in1=m1[:, :nt], op=AL.max)
                    j = 0
                    while j * 128 < nt:
                        wdx = (c0 + j * 128) // 128
                        ap2 = tps2.tile([128, 260], f32, tag="p2")
                        nc.tensor.matmul(out=ap2[:, :4], lhsT=x1[:, j * 128:(j + 1) * 128],
                                         rhs=cd_sb[:], start=True, stop=True)
                        nc.vector.tensor_copy(a_dst[:, wdx, :], ap2[:, :4])
                        j += 1

            tc.strict_bb_all_engine_barrier()

            # ---------------- phase E: edges (+ inline finalize) -----------
            with tc.tile_pool(name="pe_g", bufs=4) as gp, \
                 tc.tile_pool(name="pe_i", bufs=3) as ip, \
                 tc.tile_pool(name="pe_s", bufs=3) as sp, \
                 tc.tile_pool(name="pe_wp", bufs=2, space="PSUM") as wp, \
                 tc.tile_pool(name="pe_ap", bufs=2, space="PSUM") as app, \
                 tc.tile_pool(name="pf_ps", bufs=1, space="PSUM") as fps:
                bn_s = fps.tile([1, D], f32, tag="bns")
                bn_q = fps.tile([1, D], f32, tag="bnq")

                def finalize_window(wdx):
                    dn = sp.tile([128, 4], f32, tag="dn", name=f"dn{wdx}")
                    nc.vector.tensor_scalar_add(dn[:], slab[:, wdx, 256:260], 1e-30)
                    rd = sp.tile([128, 4], f32, tag="rd", name=f"rd{wdx}")
                    nc.vector.reciprocal(rd[:], dn[:])
                    tt = sp.tile([128, 256], f32, tag="tt", name=f"tt{wdx}")
                    nc.vector.tensor_tensor(
                        out=tt[:].rearrange("p (h d) -> p h d", h=4),
                        in0=slab[:, wdx, :256].rearrange("p (h d) -> p h d", h=4),
                        in1=rd[:].broadcast_to([128, 4, 64]),
                        op=AL.mult)
                    t2 = sp.tile([128, 128], f32, tag="t2", name=f"t2{wdx}")
                    nc.vector.tensor_tensor(out=t2[:], in0=tt[:, :128], in1=tt[:, 128:],
                                            op=AL.add)
                    y1 = sp.tile([128, D], f32, tag="y1", name=f"y1{wdx}")
                    nc.vector.tensor_tensor(out=y1[:], in0=t2[:, :64], in1=t2[:, 64:],
                                            op=AL.add)
                    nc.vector.scalar_tensor_tensor(
                        out=y_sb[:, wdx, :], in0=y1[:], scalar=0.25, in1=bias_sb[:],
                        op0=AL.mult, op1=AL.add)
                    sq = sp.tile([128, D], f32, tag="sq", name=f"sq{wdx}")
                    nc.scalar.square(sq[:], y_sb[:, wdx, :])
                    msk = ones_sb if wdx < NW - 1 else rmask_sb
                    nc.tensor.matmul(out=bn_s[:], lhsT=msk[:], rhs=y_sb[:, wdx, :],
                                     start=(wdx == 0), stop=(wdx == NW - 1))
                    nc.tensor.matmul(out=bn_q[:], lhsT=msk[:], rhs=sq[:],
                                     start=(wdx == 0), stop=(wdx == NW - 1))

                wpt_by_sec = {}
                dpt_by_sec = {}
                for ci in range(NCALLS):
                    kind = sched["call_kind"][ci]
                    tbl = table[0:LO + 1, :] if kind == 0 else table[LO + 1:2 * LO + 2, :]
                    gt = gp.tile([128, BPC, RE], bf16, tag="g")
                    nc.gpsimd.dma_gather(
                        out_ap=gt[:], in_ap=tbl,
                        idxs_ap=gidx_sb[:, ci * 64:(ci + 1) * 64],
                        num_idxs=CALL, num_idxs_reg=CALL, elem_size=RE,
                        queue_num=ci % 4)
                    indall = ip.tile([128, BPC * 256], fp8, tag="ind")
                    nc.sync.dma_start(indall[:], INDB[:, ci * 2048:(ci + 1) * 2048])
                    ind_t = indall[:, :BPC * 128]
                    indt_t = indall[:, BPC * 128:]

                    adst_pt = app.tile([128, BPC, 4], f32, tag="adst")
                    live = []
                    for b in range(BPC):
                        info = binfo[ci * BPC + b]
                        if info["dead"]:
                            continue
                        live.append((b, info))
                        nc.tensor.matmul(
                            out=adst_pt[:, b, :],
                            lhsT=indt_t[:, b * 128:(b + 1) * 128],
                            rhs=a_dst[:, info["w"], :],
                            start=True, stop=True)
                    if not live:
                        continue
                    e0 = sp.tile([128, BPC, 16], f32, tag="e0")
                    nc.vector.tensor_tensor(
                        out=e0[:, :, :4], in0=gt[:].bitcast(f32)[:, :, 128:132],
                        in1=adst_pt[:], op=AL.add)
                    e1 = sp.tile([128, BPC, 16], f32, tag="e1")
                    nc.vector.scalar_tensor_tensor(
                        out=e1[:], in0=e0[:], scalar=NEG, in1=e0[:],
                        op0=AL.mult, op1=AL.max)
                    pb = sp.tile([128, BPC, 16], bf16, tag="pb")
                    nc.scalar.activation(pb[:].rearrange("p a b -> p (a b)"),
                                         e1[:].rearrange("p a b -> p (a b)"), AF.Exp)
                    msg = sp.tile([128, BPC, 256], bf16, tag="msg")
                    nc.vector.tensor_tensor(
                        out=msg[:].rearrange("p c (h d) -> p c h d", h=4),
                        in0=gt[:, :, :256].rearrange("p c (h d) -> p c h d", h=4),
                        in1=pb[:, :, :4].broadcast_to([128, BPC, 4, 64]),
                        op=AL.mult)
                    for b, info in live:
                        s = info["sec"]
                        if info["start"]:
                            wpt_by_sec[s] = wp.tile([128, 256], f32, tag="wpt", name=f"wpt{s}")
                            dpt_by_sec[s] = app.tile([128, 4], f32, tag="dpt", name=f"dpt{s}")
                        nc.tensor.matmul(
                            out=wpt_by_sec[s][:],
                            lhsT=ind_t[:, b * 128:(b + 1) * 128],
                            rhs=msg[:, b, :],
                            start=info["start"], stop=info["stop"])
                        nc.tensor.matmul(
                            out=dpt_by_sec[s][:],
                            lhsT=ind_t[:, b * 128:(b + 1) * 128],
                            rhs=pb[:, b, :4],
                            start=info["start"], stop=info["stop"])
                        if info["stop"]:
                            wdx = info["w"]
                            if info["kind"] == 0:
                                nc.scalar.copy(slab[:, wdx, :256], wpt_by_sec[s][:])
                                nc.vector.tensor_copy(slab[:, wdx, 256:260], dpt_by_sec[s][:])
                            else:
                                nc.vector.tensor_tensor(
                                    out=slab[:, wdx, :256], in0=slab[:, wdx, :256],
                                    in1=wpt_by_sec[s][:], op=AL.add)
                                nc.vector.tensor_tensor(
                                    out=slab[:, wdx, 256:260], in0=slab[:, wdx, 256:260],
                                    in1=dpt_by_sec[s][:], op=AL.add)
                            del wpt_by_sec[s]
                            del dpt_by_sec[s]
                            if info["kind"] == 1:
                                finalize_window(wdx)

                # ---------------- phase B: BN + relu + store ---------------
                fp_ = sp
                st = fp_.tile([1, 128], f32, tag="st")
                nc.vector.tensor_copy(st[:, :64], bn_s[:])
                nc.vector.tensor_copy(st[:, 64:], bn_q[:])
                nc.gpsimd.dma_start(cc_in[:], st[:])
                nc.gpsimd.collective_compute(
                    "AllReduce", AL.add, replica_groups=[list(range(NC))],
                    ins=[cc_in[:].opt()], outs=[cc_out[:].opt()])
                st2 = fp_.tile([1, 128], f32, tag="st2")
                nc.gpsimd.dma_start(st2[:], cc_out[:])
                mean = fp_.tile([1, D], f32, tag="mean")
                nc.vector.tensor_scalar_mul(mean[:], st2[:, :64], 1.0 / N)
                ex2 = fp_.tile([1, D], f32, tag="ex2")
                nc.vector.tensor_scalar_mul(ex2[:], st2[:, 64:], 1.0 / N)
                msq = fp_.tile([1, D], f32, tag="msq")
                nc.scalar.square(msq[:], mean[:])
                var = fp_.tile([1, D], f32, tag="var")
                nc.vector.tensor_tensor(out=var[:], in0=ex2[:], in1=msq[:],
                                        op=AL.subtract)
                veps = fp_.tile([1, D], f32, tag="veps")
                nc.vector.tensor_scalar_add(veps[:], var[:], BN_EPS)
                sd = fp_.tile([1, D], f32, tag="sd")
                nc.scalar.sqrt(sd[:], veps[:])
                rs = fp_.tile([1, D], f32, tag="rs")
                nc.vector.reciprocal(rs[:], sd[:])
                scsh = fp_.tile([1, 128], f32, tag="scsh")
                nc.vector.tensor_tensor(out=scsh[:, :64], in0=gb_sb[:, :64], in1=rs[:],
                                        op=AL.mult)
                mssc = fp_.tile([1, D], f32, tag="mssc")
                nc.vector.tensor_tensor(out=mssc[:], in0=mean[:], in1=scsh[:, :64],
                                        op=AL.mult)
                nc.vector.tensor_tensor(out=scsh[:, 64:], in0=gb_sb[:, 64:], in1=mssc[:],
                                        op=AL.subtract)
                bc = sp.tile([128, 128], f32, tag="bc")
                nc.gpsimd.partition_broadcast(bc[:], scsh[:])
                for wdx in range(NW):
                    z = sp.tile([128, D], f32, tag="z", name=f"z{wdx}")
                    nc.vector.tensor_tensor(out=z[:], in0=y_sb[:, wdx, :],
                                            in1=bc[:, :64], op=AL.mult)
                    z2 = sp.tile([128, D], f32, tag="z2", name=f"z2{wdx}")
                    nc.vector.tensor_tensor(out=z2[:], in0=z[:], in1=bc[:, 64:],
                                            op=AL.add)
                    zo = sp.tile([128, D], f32, tag="zo", name=f"zo{wdx}")
                    nc.scalar.activation(zo[:], z2[:], AF.Relu)
                    rows = W if wdx < NW - 1 else LAST_ROWS
                    nc.sync.dma_start(out_slab[wdx * W:wdx * W + rows, :], zo[:rows, :])

    nc.compile()
    return nc


def kernel(x, edge_index, W_lin, b_lin, prelu_w, W_gat, att_src, att_dst,
           gat_bias, bn_gamma, bn_beta):
    global LAST_EXEC_NS, LAST_TRACE
    from concourse import bass_utils

    x = np.asarray(x, np.float32)
    edge_index = np.asarray(edge_index)
    W_lin = np.asarray(W_lin, np.float32)
    b_lin = np.asarray(b_lin, np.float32)
    prelu_w = np.asarray(prelu_w, np.float32)
    W_gat = np.asarray(W_gat, np.float32)
    att_src = np.asarray(att_src, np.float32)
    att_dst = np.asarray(att_dst, np.float32)
    gat_bias = np.asarray(gat_bias, np.float32)
    bn_gamma = np.asarray(bn_gamma, np.float32)
    bn_beta = np.asarray(bn_beta, np.float32)

    key = hashlib.sha1(np.ascontiguousarray(edge_index).tobytes()).hexdigest()
    if key not in _CACHE:
        sched, blobs = _schedule_and_blobs(edge_index)
        nc = _build_program(sched)
        _CACHE[key] = (sched, blobs, nc)
    sched, blobs, nc = _CACHE[key]

    # b_lin is zero in the reference setup; if nonzero, do the pre-linear
    # exactly on host and feed the device an identity pre-stage.
    if np.any(b_lin != 0):
        x1_host = x @ W_lin.T + b_lin
        x1_host = np.where(x1_host >= 0, x1_host, prelu_w * x1_host)
        # then device treats W_lin as identity and prelu as identity:
        xT_eff = np.ascontiguousarray(x1_host.T)
        W1_eff = np.eye(64, dtype=np.float32)
        prelu_eff = np.ones((64,), np.float32)
    else:
        xT_eff = np.ascontiguousarray(x.T)
        W1_eff = W_lin
        prelu_eff = prelu_w

    C_src = np.zeros((64, 4), np.float32)
    C_dst = np.zeros((64, 4), np.float32)
    for h in range(H):
        Wh = W_gat[h * 64:(h + 1) * 64, :]  # [64, 64] maps x1 -> head h
        C_src[:, h] = Wh.T @ att_src[h]
        C_dst[:, h] = Wh.T @ att_dst[h]

    bf = ml_dtypes.bfloat16
    W1T_np = np.ascontiguousarray(W1_eff.T).astype(bf)  # [din, dout]
    WC_np = np.concatenate([np.ascontiguousarray(W_gat.T), C_src], axis=1).astype(bf)
    CD_np = C_dst.astype(bf)
    xT_bf = xT_eff.astype(bf)

    rmask = np.zeros((128, 1), np.float32)
    rmask[:LAST_ROWS] = 1.0

    in_maps = []
    for c in range(NC):
        xs = np.zeros((64, NW * 128), np.float32)
        xs[:, :SLAB] = xT_eff[:, c * SLAB:(c + 1) * SLAB]
        in_maps.append(dict(
            xT=xT_bf,
            xTs=xs.astype(bf),
            W1T=W1T_np, WC=WC_np, CD=CD_np,
            prelu=prelu_eff.reshape(64, 1),
            GIDX=blobs[c]["GIDX"], INDB=blobs[c]["INDB"],
            bias128=np.tile(gat_bias[None, :], (128, 1)),
            ones_col=np.ones((128, 1), np.float32),
            rmask_col=rmask,
            onesrow=np.ones((1, 128), np.float32),
            gb_row=np.concatenate([bn_gamma, bn_beta])[None, :],
        ))

    trace = os.environ.get("GAT_TRACE", "0") == "1"
    if trace:
        _install_ntff_shim()
    res = bass_utils.run_bass_kernel_spmd(nc, in_maps, core_ids=list(range(NC)),
                                          trace=trace)
    LAST_EXEC_NS = res.exec_time_ns
    LAST_TRACE = res.instructions_and_trace
    out = np.empty((N, D), np.float32)
    for c in range(NC):
        out[c * SLAB:(c + 1) * SLAB] = res.results[c]["out_slab"]
    return out
